# revision 25
# baseline (speedup 1.0000x reference)
"""Locally-connected 2D conv (unshared weights), VALID, stride 2 — Trainium2 Bass kernel.

Problem (hardcoded):
  x:       (16, 32, 113, 113) f32
  weights: (56, 56, 32, 3, 3, 64) f32   (H_out, W_out, C_in, kh, kw, C_out)
  bias:    (56, 56, 64) f32
  out:     (16, 64, 56, 56) f32
  out[b,o,u,v] = sum_{c,q,r} x[b,c,2u+q,2v+r] * weights[u,v,c,q,r,o] + bias[u,v,o]

Sharding: H_out split across 8 cores (7 output rows each); each core reads only
its 1/8 of the weight tensor (the dominant traffic).

Design (driven by traces: DMA engines cap at ~13.4 GB/s each / ~214 GB/s
aggregate; tensor-engine matmul instruction overhead ~70-110ns):
  - weights and x are cast to bf16 on host: halves the dominant HBM traffic
    (rel-err budget is 2e-2; bf16 contributes ~4e-3).
  - x pack stores only the rows actually read: partition p = q*32+c holds rows
    2t+q (t=0..6 -> output row u=t), i.e. 7 rows per q-shift instead of the
    full 13-row band. Two tiles (t<4 / t>=4) so matmuls don't wait on the
    full transfer.
  - v-pair matmuls: lhsT = [96, 128] = weights of two adjacent output columns
    (vsel, o); rhs = [96, (vsel, b)] x column pair (one AP thanks to W padded
    113->114 and (w2, parity) packing). 588 matmuls instead of 1176; the
    off-diagonal psum blocks are discarded.
  - weights stream per (u, ch) chunk ([96, 5376] bf16, 10.75KB runs),
    alternating the sync/gpsimd DMA queues; x/bias/y ride the scalar queue.
  - output staged as bf16 in (u, b, v) layout -> contiguous multi-KB DMA runs,
    written back in two pieces (u<4 mid-kernel, rest at the end); host
    transposes to NCHW and upcasts.
Per-(u,ch): one PSUM accumulation group [128, 14*32] spanning 42 matmuls
(start on first, stop on last; first write per byte range overwrites). DVE
extracts the two diagonal [64, b, vp] blocks with bias add (f32 psum -> bf16).
"""

import numpy as np
import ml_dtypes

BF16 = ml_dtypes.bfloat16

B = 16
C_IN = 32
C_OUT = 64
H_OUT = 56
W_OUT = 56
KK = 3
STRIDE = 2
H_IN = 113
W_PAD = 114           # padded input row width (one zero col)
W2 = W_PAD // 2       # 57

N_CORES = 8
U_PER = H_OUT // N_CORES          # 7 output rows per core
ROWS_IN = (U_PER - 1) * STRIDE + KK  # 15 input rows per core
KPART = C_IN * KK                 # 96 contraction partitions (q,c)
VP = 14                           # v-pairs per PSUM chunk
NCH = 2                           # chunks per u  (2*14*2 = 56 = W_OUT)
WFREE_CH = VP * KK * 2 * C_OUT    # weight free per (u,ch) chunk (5376)
U_LO = 4                          # u rows in first x/output tile
ROW_ELEMS = B * W_PAD             # 1824

_CACHE = {}


def _build():
    import concourse.mybir as mybir
    from concourse import bacc
    from concourse.tile import TileContext

    f32 = mybir.dt.float32
    bf16 = mybir.dt.bfloat16
    nc = bacc.Bacc("TRN2", target_bir_lowering=False, debug=False,
                   num_devices=N_CORES)
    # Host-prepacked tensors (see _pack_core):
    #   xp[p, (t*16+b)*114 + w] = x[b, c, 2*(u0+t)+q, w],  p = q*32+c, w<113
    #   wp[u, ch, p, ((vp*3+r)*2+vs)*64+o] = weights[u0+u, ch*28+vp*2+vs, c, q, r, o]
    #   bp[o, ((u*2+ch)*14+vp)*2+vs] = bias[u0+u, ch*28+vp*2+vs, o]
    #   y[o, (u*16+b)*56 + v] (bf16)
    xp_in = nc.dram_tensor("xp", [KPART, U_PER * ROW_ELEMS], bf16,
                           kind="ExternalInput").ap()
    wp_in = nc.dram_tensor("wp", [U_PER, NCH, KPART, WFREE_CH], bf16,
                           kind="ExternalInput").ap()
    bp_in = nc.dram_tensor("bp", [C_OUT, U_PER * W_OUT], f32,
                           kind="ExternalInput").ap()
    y_out = nc.dram_tensor("y", [C_OUT, U_PER * B * W_OUT], bf16,
                           kind="ExternalOutput").ap()

    xsplit = U_LO * ROW_ELEMS     # x free elems for u<4
    ysplit = U_LO * B * W_OUT     # y free elems for u<4

    with TileContext(nc) as tc:
        with tc.tile_pool(name="xpool", bufs=1) as xpool, \
             tc.tile_pool(name="wpool", bufs=5) as wpool, \
             tc.tile_pool(name="opool", bufs=1) as opool, \
             tc.tile_pool(name="pspool", bufs=8, space="PSUM") as pspool:

            # scalar queue: bias FIRST — the first PSUM eviction (DVE) needs
            # it at ~23us, and behind x it would land ~40us, stalling psum
            # tile recycling and with it the matmul pipeline
            bt = xpool.tile([C_OUT, U_PER * W_OUT], f32)
            nc.scalar.dma_start(out=bt[:], in_=bp_in[:])
            xa = xpool.tile([KPART, xsplit], bf16)
            nc.scalar.dma_start(out=xa[:], in_=xp_in[:, :xsplit])
            xb = xpool.tile([KPART, (U_PER - U_LO) * ROW_ELEMS], bf16)
            nc.scalar.dma_start(out=xb[:], in_=xp_in[:, xsplit:])

            oa1 = opool.tile([C_OUT, ysplit], bf16)
            oa2 = opool.tile([C_OUT, (U_PER - U_LO) * B * W_OUT], bf16)

            # x views: [p, pr, t, w2, b]; w = w2*2 + pr, t = local output row
            xva = xa.rearrange("p (t b w2 pr) -> p pr t w2 b",
                               t=U_LO, b=B, w2=W2, pr=2)
            xvb = xb.rearrange("p (t b w2 pr) -> p pr t w2 b",
                               t=U_PER - U_LO, b=B, w2=W2, pr=2)
            # bias view: [o, u, ch, vs, vp]
            bv = bt.rearrange("p (u ch vp vs) -> p u ch vs vp",
                              u=U_PER, ch=NCH, vp=VP, vs=2)

            for u in range(U_PER):
                xv = xva if u < U_LO else xvb
                xtl = u if u < U_LO else u - U_LO
                otl = xtl
                oa = oa1 if u < U_LO else oa2
                ov = oa.rearrange("p (u b ch vp vs) -> p u ch vs b vp",
                                  u=(U_LO if u < U_LO else U_PER - U_LO),
                                  b=B, ch=NCH, vp=VP, vs=2)
                for ch in range(NCH):
                    weng = nc.sync if (u * NCH + ch) % 2 == 0 else nc.gpsimd
                    # final chunk: asymmetric split (10 vp + 4 vp) into
                    # separate tiles/psum groups so only the small remainder
                    # computes after the last weight byte lands
                    if u == U_PER - 1 and ch == NCH - 1:
                        vp_groups = [(0, 10), (10, VP)]
                    else:
                        vp_groups = [(0, VP)]
                    for vp0, vp1 in vp_groups:
                        nvp = vp1 - vp0
                        fpv = KK * 2 * C_OUT   # weight elems per vp (384)
                        wt = wpool.tile([KPART, nvp * fpv], bf16)
                        weng.dma_start(
                            out=wt[:],
                            in_=wp_in[u, ch, :, vp0 * fpv:vp1 * fpv])
                        # weight view: [p, vp, r, (vs o)]
                        wv = wt.rearrange("p (vp r vs o) -> p vp r (vs o)",
                                          vp=nvp, r=KK, vs=2, o=C_OUT)
                        ps = pspool.tile([2 * C_OUT, nvp * 32], f32)
                        for vp in range(vp0, vp1):
                            for r in range(KK):
                                v = ch * 2 * VP + vp * 2
                                w2 = v + r // 2
                                rhs = xv[:, r % 2, xtl, w2:w2 + 2, :]
                                nc.tensor.matmul(
                                    ps[:, (vp - vp0) * 32:(vp - vp0 + 1) * 32],
                                    wv[:, vp - vp0, r], rhs,
                                    start=(vp == vp0 and r == 0),
                                    stop=(vp == vp1 - 1 and r == KK - 1),
                                )
                        # psum view: [vs*64+o, b, vs', vp]; diagonal blocks
                        psv = ps.rearrange("p (vp vs b) -> p b vs vp",
                                           vp=nvp, vs=2, b=B)
                        for vs in range(2):
                            nc.vector.tensor_add(
                                ov[:, otl, ch, vs, :, vp0:vp1],
                                psv[vs * C_OUT:(vs + 1) * C_OUT, :, vs, :],
                                bv[:, u, ch, vs, vp0:vp1].unsqueeze(
                                    1).broadcast_to([C_OUT, B, nvp]))
                if u == U_LO - 1:
                    nc.scalar.dma_start(out=y_out[:, :ysplit], in_=oa1[:])
            nc.scalar.dma_start(out=y_out[:, ysplit:], in_=oa2[:])

    nc.compile()
    return nc


def _get_nc():
    if "nc" not in _CACHE:
        _CACHE["nc"] = _build()
    return _CACHE["nc"]


def _pack_core(x, weights, bias, i):
    u0 = i * U_PER
    # x': (96, 7*16*114); p = q*32+c holds rows 2*(u0+t)+q; free (t, b, w)
    xs = x[:, :, STRIDE * u0:STRIDE * u0 + ROWS_IN, :]      # (B, C, 15, 113)
    xq = np.stack([xs[:, :, q:q + 2 * U_PER - 1:2, :] for q in range(KK)],
                  axis=0)                                   # (q, B, C, 7, 113)
    xq = xq.transpose(0, 2, 3, 1, 4)                        # (q, c, t, b, w)
    xp = np.zeros((KPART, U_PER, B, W_PAD), dtype=BF16)
    xp[:, :, :, :H_IN] = xq.reshape(KPART, U_PER, B, H_IN)
    xp = xp.reshape(KPART, U_PER * ROW_ELEMS)

    # w': (7, 2, 96, 5376); p = q*32+c, free (vp, r, vs, o)
    ws = weights[u0:u0 + U_PER].reshape(U_PER, NCH, VP, 2, C_IN, KK, KK,
                                        C_OUT)                # u ch vp vs c q r o
    ws = ws.transpose(0, 1, 5, 4, 2, 6, 3, 7)                 # u ch q c vp r vs o
    wp = np.ascontiguousarray(
        ws.reshape(U_PER, NCH, KPART, WFREE_CH)).astype(BF16)

    # b': (64, 392): bp[o, u*56+v] f32
    bp = np.ascontiguousarray(
        bias[u0:u0 + U_PER].reshape(U_PER * W_OUT, C_OUT).T)
    return {"xp": xp, "wp": wp, "bp": bp}


def kernel(x, weights, bias, _trace=False, _tmpdir=None):
    from concourse.bass_utils import run_bass_kernel_spmd

    x = np.ascontiguousarray(x, dtype=np.float32)
    weights = np.ascontiguousarray(weights, dtype=np.float32)
    bias = np.ascontiguousarray(bias, dtype=np.float32)

    nc = _get_nc()
    core_ids = list(range(N_CORES))
    in_maps = [_pack_core(x, weights, bias, i) for i in core_ids]
    res = run_bass_kernel_spmd(nc, in_maps, core_ids, trace=_trace,
                               tmpdir=_tmpdir)
    parts = []
    for i in core_ids:
        y = np.asarray(res.results[i]["y"]).astype(np.float32)
        # (o, u, b, v) -> (b, o, u, v)
        parts.append(y.reshape(C_OUT, U_PER, B, W_OUT).transpose(2, 0, 1, 3))
    out = np.concatenate(parts, axis=2)
    if _trace:
        _CACHE["last_result"] = res
    return out
